# revision 44
# baseline (speedup 1.0000x reference)
"""Multi-head self-attention (RoPE, causal) Trainium2 Bass kernel.

Sharding: head-parallel across 8 NeuronCores. Core c owns heads {2c, 2c+1}
for both batch rows. Each core computes its heads' QKV projection, RoPE,
causal flash attention (scores kept transposed [k, q]), the per-head
softmax normalization, and a partial output projection against its 128
columns of W_o. The host sums the 8 partial projections (the "all-reduce")
and reshapes.

v2 scheduling: per-partition-contiguous DRAM layouts (fast DMA), QKV
processed in 1024-column chunks with RoPE fused per chunk, batch-0 scores
emitted while batch-1 QKV still projects, and the two batches' attention
software-pipelined so ACT (exp) and PE (matmul) stay saturated; f32r for
the reciprocal-broadcast matmul; V-augmentation built by the transpose DMA
itself.

Self-contained: hardcodes B=2, S=2048, D=1024, H=16, d_k=64.
"""
import numpy as np
import ml_dtypes

B, S, D, H, DK = 2, 2048, 1024, 16, 64
NCORES = 8
HPC = H // NCORES            # 2 heads per core
THETA = 10000.0
BS = B * S                   # 4096 flattened tokens (b-major)
KT = D // 128                # 8 contraction tiles
P = 128
NCH = 4                      # x column chunks of 1024

bf16 = ml_dtypes.bfloat16

_CACHED_NC = None


def _host_prep(x, token_positions, W_qkv, W_o):
    """Build per-core DRAM input dicts (numpy, bf16), SBUF-layout contiguous."""
    cast = lambda a: np.ascontiguousarray(a).astype(bf16)
    X2T = np.asarray(x, np.float32).reshape(BS, D).T          # [D, BS]
    xt = cast(X2T.reshape(KT, P, NCH, 1024).transpose(1, 2, 0, 3))  # [P,4,KT,1024]

    pos = np.asarray(token_positions, np.float64)
    inv = THETA ** (-np.arange(0, DK, 2, dtype=np.float64) / DK)   # [32]
    ang = pos[:, None] * inv[None, :]                              # [S, 32]
    cosv = np.cos(ang).T.astype(np.float32)                        # [32, S]
    sinv = np.sin(ang).T.astype(np.float32)
    COS = cast(np.tile(cosv, (4, 1)))                              # [128, S]
    SINS = cast(np.concatenate([-sinv, sinv, -sinv, sinv], 0))     # [128, S]

    perm = np.concatenate([np.arange(0, 64, 2), np.arange(1, 64, 2)])
    tri = cast(np.triu(np.ones((P, P), np.float32)))               # [k,q]: q>=k

    sel4 = np.zeros((4, 4, 64), np.float32)
    for u in range(4):
        sel4[u, u, :] = 1.0                                        # lhsT rows=K

    def wlayout(w):        # [D, 128] -> [P, KT, 128] per-partition contiguous
        return cast(w.T.reshape(KT, P, P).transpose(1, 0, 2))

    Wqkv = np.asarray(W_qkv, np.float32)
    Wo = np.asarray(W_o, np.float32)
    maps = []
    for c in range(NCORES):
        hA = HPC * c
        rows = np.concatenate([(hA + 0) * 64 + perm, (hA + 1) * 64 + perm])
        rows_v = np.concatenate([(hA + 0) * 64 + np.arange(64),
                                 (hA + 1) * 64 + np.arange(64)])
        maps.append({
            "xt": xt,
            "wq": wlayout(Wqkv[rows]),
            "wk": wlayout(Wqkv[D + rows]),
            "wv": wlayout(Wqkv[2 * D + rows_v]),
            "wo": cast(Wo[:, P * c:P * c + P].T),                  # [128, 1024]
            "cos": COS,
            "sin": SINS,
            "tri": tri,
            "sel4": sel4,
        })
    return maps


def _build_nc():
    """Trace + compile the per-core Bass module (same program on all cores)."""
    from contextlib import ExitStack
    import concourse.bacc as bacc
    import concourse.mybir as mybir
    import concourse.tile as tile
    from concourse.bass import ts

    f32 = mybir.dt.float32
    f32r = mybir.dt.float32r
    bf = mybir.dt.bfloat16
    EXP = mybir.ActivationFunctionType.Exp

    nc = bacc.Bacc("TRN2", target_bir_lowering=False, debug=False,
                   enable_asserts=False)

    xt_d = nc.dram_tensor("xt", [P, NCH, KT, 1024], bf, kind="ExternalInput").ap()
    wq_d = nc.dram_tensor("wq", [P, KT, P], bf, kind="ExternalInput").ap()
    wk_d = nc.dram_tensor("wk", [P, KT, P], bf, kind="ExternalInput").ap()
    wv_d = nc.dram_tensor("wv", [P, KT, P], bf, kind="ExternalInput").ap()
    wo_d = nc.dram_tensor("wo", [P, D], bf, kind="ExternalInput").ap()
    cos_d = nc.dram_tensor("cos", [P, S], bf, kind="ExternalInput").ap()
    sin_d = nc.dram_tensor("sin", [P, S], bf, kind="ExternalInput").ap()
    tri_d = nc.dram_tensor("tri", [P, P], bf, kind="ExternalInput").ap()
    sel_d = nc.dram_tensor("sel4", [4, 4, 64], f32r, kind="ExternalInput").ap()
    yt_d = nc.dram_tensor("yt", [P, 8, 8, 512], bf, kind="ExternalOutput").ap()

    with tile.TileContext(nc) as tc, ExitStack() as ctx:
        # ---- kernel-lifetime tiles / pools ----
        pp = ctx.enter_context(tc.tile_pool(name="persist", bufs=1))
        WO = pp.tile([P, D], bf, tag="wo")
        TRI = pp.tile([P, P], bf, tag="tri")
        SEL4 = pp.tile([4, 4, 64], f32r, tag="sel4")
        SCR = pp.tile([P, 8], bf, tag="scr")
        # attention working buffers
        ab = ctx.enter_context(tc.tile_pool(name="attnbuf", bufs=1))
        QA = ab.tile([P, BS], bf, tag="qa")
        KA = ab.tile([P, BS], bf, tag="ka")
        VT = ab.tile([P, BS], bf, tag="vt")
        VAB = [ab.tile([P, 16, 132], bf, tag=f"vab{b}", name=f"vab{b}")
               for b in range(B)]
        # cycling pools (slots shared per tag)
        oap = ctx.enter_context(tc.tile_pool(name="oaccp", bufs=1))
        srp = ctx.enter_context(tc.tile_pool(name="sums", bufs=1))
        opp = ctx.enter_context(tc.tile_pool(name="oprp", bufs=2))
        stg = ctx.enter_context(tc.tile_pool(name="stage", bufs=2))
        ybp = ctx.enter_context(tc.tile_pool(name="ybig", bufs=2))
        ptA0 = ctx.enter_context(tc.tile_pool(name="pta0", bufs=8))
        ptA1 = ctx.enter_context(tc.tile_pool(name="pta1", bufs=8))
        ptB0 = ctx.enter_context(tc.tile_pool(name="ptb0", bufs=8))
        ptB1 = ctx.enter_context(tc.tile_pool(name="ptb1", bufs=8))
        scp = ctx.enter_context(tc.tile_pool(name="scps", bufs=2, space="PSUM"))

        nc.gpsimd.memset(SCR[:], 0.0)

        pts = {}
        OACC = [None, None]

        def scores_group(b, g, pools=None, irange=None):
            for i in irange or range(8 * g, 8 * g + 8):
                qs_i = 512 * (i // 4)
                qext = S - qs_i
                blk = b * S + 128 * i
                for h in range(HPC):
                    hsl = slice(64 * h, 64 * h + 64)
                    pool = (pools or (ptA0, ptA1, ptB0, ptB1))[i // 4]
                    ptw = (2048, 1536, 1024, 512)[i // 4]
                    pt = pool.tile([P, ptw], bf, tag="pt")
                    off = 0
                    while off < qext:
                        w = min(1024, qext - off)
                        ps = scp.tile([P, 1024], f32, tag="sc")
                        vf = max(0, 128 * i - (qs_i + off))
                        for qc in range(0, w, 512):
                            sub = min(512, w - qc)
                            q0 = qs_i + off + qc
                            nc.tensor.matmul(
                                ps[:, qc:qc + sub],
                                lhsT=KA[hsl, blk:blk + 128],
                                rhs=QA[hsl, b * S + q0:b * S + q0 + sub],
                                start=True, stop=True)
                        if vf < w:
                            nc.scalar.activation(
                                pt[:, off + vf:off + w],
                                ps[:, vf:w], EXP, scale=0.125)
                        if vf > 0:
                            nc.gpsimd.memset(pt[:, off:off + vf], 0.0)
                        off += w
                    dc = 128 * i - qs_i
                    nc.gpsimd.tensor_mul(pt[:, dc:dc + 128],
                                         pt[:, dc:dc + 128], TRI[:])
                    pts[(b, i, h)] = pt

        def emit_av(b, jlist, avp):
            # one PSUM accumulation group per (j, h) over all causal k-blocks
            if OACC[b] is None:
                OACC[b] = oap.tile([65, 8, 512], f32, tag="oacc",
                                   name=f"oacc{b}")
            for j in jlist:
                ilist = [i for i in range(16) if i <= 4 * j + 3]
                for h in range(HPC):
                    pa = avp.tile([65, 512], f32, tag="av")
                    for n, i in enumerate(ilist):
                        qs_i = 512 * (i // 4)
                        o0 = 512 * j - qs_i
                        nc.tensor.matmul(
                            pa[:],
                            lhsT=VAB[b][:, i, 65 * h:65 * h + 65],
                            rhs=pts[(b, i, h)][:, o0:o0 + 512],
                            start=(n == 0),
                            stop=(n == len(ilist) - 1))
                    u8 = j * 2 + h
                    nc.vector.tensor_copy(OACC[b][:, u8, :], pa[:])

        def div_oproj(b, jlist, avp, yps, tail=False):
            u0 = 2 * jlist[0]
            R = 2 * len(jlist)
            SUt = srp.tile([4, 512], f32, tag="sums")
            REf = srp.tile([4, 512], f32, tag="recip")
            REtt = srp.tile([4, 512], f32r, tag="recipr")
            SU, RE, REt = SUt[0:R, :], REf[0:R, :], REtt[0:R, :]
            nc.gpsimd.dma_start(SU, OACC[b][64:65, u0:u0 + R, :])
            nc.vector.reciprocal_approx_fast(RE, SU)
            nc.vector.tensor_copy(REt, RE)
            REr = REt
            for j in jlist:
                jj = b * 4 + j
                OPR = opp.tile([P, 512], bf, tag="opr")
                for h in range(HPC):
                    u8 = j * 2 + h
                    u4 = 2 * (j - jlist[0]) + h
                    pbt = avp.tile([65, 512], f32, tag="av", name="pbt")
                    pb = pbt[0:64, :]
                    nc.tensor.matmul(pb, lhsT=SEL4[0:R, u4, :], rhs=REr,
                                     start=True, stop=True)
                    if h == 0:
                        nc.vector.tensor_mul(OPR[0:64, :],
                                             OACC[b][0:64, u8, :], pb)
                    else:
                        tb = stg.tile([64, 512], bf, tag="tmpb")
                        nc.vector.tensor_mul(tb[:],
                                             OACC[b][0:64, u8, :], pb)
                        nc.gpsimd.dma_start(OPR[64:128, :], tb[:])
                for eh in range(2):
                    yb = ybp.tile([P, 4, 512], bf, tag="yb")
                    for e4 in range(4):
                        et = eh * 4 + e4
                        py = yps.tile([P, 512], f32, tag="y")
                        nc.tensor.matmul(py[:], lhsT=WO[:, ts(et, P)],
                                         rhs=OPR[:], start=True, stop=True)
                        if tail and et % 2 == 1:
                            nc.scalar.copy(yb[:, e4, :], py[:])
                        else:
                            nc.vector.tensor_copy(yb[:, e4, :], py[:])
                    nc.sync.dma_start(yt_d[:, jj, 4 * eh:4 * eh + 4, :], yb[:])

        # ---- QKV phase: 1024-col chunks, rope fused per chunk ----
        with tc.tile_pool(name="xp", bufs=2) as xp, \
             tc.tile_pool(name="wp", bufs=1) as wp, \
             tc.tile_pool(name="swap", bufs=2) as swp, \
             tc.tile_pool(name="vbp", bufs=2) as vbp, \
             tc.tile_pool(name="qkvps", bufs=2, space="PSUM") as qps:
            WQ = wp.tile([P, KT, P], bf, tag="wq")
            WK = wp.tile([P, KT, P], bf, tag="wk")
            WV = wp.tile([P, KT, P], bf, tag="wv")
            COS = wp.tile([P, S], bf, tag="cos")
            SIN = wp.tile([P, S], bf, tag="sin")
            nc.scalar.dma_start(WQ[:], wq_d)
            nc.scalar.dma_start(WK[:], wk_d)
            nc.scalar.dma_start(COS[:], cos_d)
            nc.scalar.dma_start(SIN[:], sin_d)
            nc.scalar.dma_start(WV[:], wv_d)
            nc.scalar.dma_start(WO[:], wo_d)
            nc.scalar.dma_start(TRI[:], tri_d)
            nc.scalar.dma_start(SEL4[:], sel_d)
            # preload the exp table set (after the weight DMAs on this queue)
            nc.scalar.activation(SCR[:], SCR[:], EXP)
            XT = []
            for c in range(NCH):
                xtc = xp.tile([P, KT, 1024], bf, tag="xt")
                # two half-chunk DMAs so the first matmuls start sooner
                nc.sync.dma_start(xtc[:, :, 0:512], xt_d[:, c, :, 0:512])
                nc.sync.dma_start(xtc[:, :, 512:1024], xt_d[:, c, :, 512:1024])
                XT.append(xtc)

            def project_chunk(Wt, DST, c, evac):
                ps = qps.tile([P, 1024], f32, tag="qkv")
                for jj in range(2):
                    for kt in range(KT):
                        nc.tensor.matmul(
                            ps[:, ts(jj, 512)], lhsT=Wt[:, kt, :],
                            rhs=XT[c][:, kt, ts(jj, 512)],
                            start=(kt == 0), stop=(kt == KT - 1))
                col = 1024 * c
                if evac == "s":
                    nc.scalar.copy(DST[:, col:col + 1024], ps[:])
                else:
                    nc.vector.tensor_copy(DST[:, col:col + 1024], ps[:])

            def rope_chunk(c):
                ssl = slice(1024 * c, 1024 * c + 1024)
                csl = slice(1024 * (c % 2), 1024 * (c % 2) + 1024)
                for A, tag in ((QA, "qs"), (KA, "ks")):
                    SWT = swp.tile([P, 1024], bf, tag=tag)
                    for blk in range(4):
                        src = blk ^ 1
                        nc.gpsimd.dma_start(SWT[32 * blk:32 * blk + 32, :],
                                            A[32 * src:32 * src + 32, ssl])
                    nc.vector.tensor_mul(A[:, ssl], A[:, ssl], COS[:, csl])
                    nc.vector.tensor_mul(SWT[:], SWT[:], SIN[:, csl])
                    nc.vector.tensor_add(A[:, ssl], A[:, ssl], SWT[:])

            def vaug(b, vbp):
                # HW xbar transpose cannot write a strided dst: go through a
                # dense tile, then gpsimd-copy into the augmented layout.
                for half, dst0 in ((0, 0), (1, 65)):
                    vb = vbp.tile([P, 16, 64], bf, tag="vb")
                    nc.sync.dma_start_transpose(
                        vb[:], VT[64 * half:64 * half + 64, b * S:(b + 1) * S])
                    nc.vector.tensor_copy(VAB[b][:, :, dst0:dst0 + 64], vb[:])
                for i in range(16):
                    nc.gpsimd.memset(VAB[b][:, i, 64:65], 1.0)
                    nc.gpsimd.memset(VAB[b][:, i, 129:130], 1.0)

            for c in (0, 1):
                project_chunk(WQ, QA, c, "s")
                project_chunk(WK, KA, c, "v")
                rope_chunk(c)
            project_chunk(WV, VT, 0, "s")
            project_chunk(WV, VT, 1, "s")
            vaug(0, vbp)
            scores_group(0, 0)            # b0 scores overlap b1 QKV
            for c in (2, 3):
                project_chunk(WQ, QA, c, "s")
                project_chunk(WK, KA, c, "v")
                rope_chunk(c)
            project_chunk(WV, VT, 2, "v")
            project_chunk(WV, VT, 3, "v")
            vaug(1, vbp)

        # ---- attention pipeline, batches interleaved ----
        with tc.tile_pool(name="avps", bufs=2, space="PSUM") as avp, \
             tc.tile_pool(name="yps", bufs=2, space="PSUM") as yps, \
             tc.tile_pool(name="pta0b", bufs=8) as ptA0b, \
             tc.tile_pool(name="pta1b", bufs=8) as ptA1b:
            emit_av(0, [0, 1], avp)        # needs only b0 g0 pts
            div_oproj(0, [0, 1], avp, yps)
            scores_group(0, 1, irange=range(8, 12))
            scores_group(1, 0, pools=(ptA0b, ptA1b, None, None))
            emit_av(0, [2], avp)
            div_oproj(0, [2], avp, yps)
            scores_group(0, 1, irange=range(12, 16))
            emit_av(0, [3], avp)
            div_oproj(0, [3], avp, yps)
            emit_av(1, [0, 1], avp)        # needs only b1 g0 pts
            div_oproj(1, [0, 1], avp, yps)
            scores_group(1, 1, irange=range(8, 12))
            emit_av(1, [2], avp)
            div_oproj(1, [2], avp, yps, tail=True)
            scores_group(1, 1, irange=range(12, 16))
            emit_av(1, [3], avp)
            div_oproj(1, [3], avp, yps, tail=True)

    nc.compile()
    return nc


def get_nc():
    global _CACHED_NC
    if _CACHED_NC is None:
        _CACHED_NC = _build_nc()
    return _CACHED_NC


def run_on_hw(in_maps, **kwargs):
    from concourse.bass_utils import run_bass_kernel_spmd
    nc = get_nc()
    return run_bass_kernel_spmd(nc, in_maps, core_ids=list(range(NCORES)),
                                **kwargs)


def _assemble(res):
    acc = np.zeros((P, 8, 8, 512), np.float32)
    for r in res.results:
        acc += np.asarray(r["yt"]).astype(np.float32)
    y = acc.transpose(2, 0, 1, 3).reshape(D, BS)     # [et,p,jj,q] -> [D, BS]
    return np.ascontiguousarray(y.T).reshape(B, S, D).astype(np.float32)


def kernel(x, token_positions, W_qkv, W_o):
    in_maps = _host_prep(x, token_positions, W_qkv, W_o)
    res = run_on_hw(in_maps)
    return _assemble(res)


# revision 45
# speedup vs baseline: 1.0715x; 1.0715x over previous
"""Multi-head self-attention (RoPE, causal) Trainium2 Bass kernel.

Sharding: head-parallel across 8 NeuronCores. Core c owns heads {2c, 2c+1}
for both batch rows. Each core computes its heads' QKV projection, RoPE,
causal flash attention (scores kept transposed [k, q]), the per-head
softmax normalization, and a partial output projection against its 128
columns of W_o. The host sums the 8 partial projections (the "all-reduce")
and reshapes.

v2 scheduling: per-partition-contiguous DRAM layouts (fast DMA), QKV
processed in 1024-column chunks with RoPE fused per chunk, batch-0 scores
emitted while batch-1 QKV still projects, and the two batches' attention
software-pipelined so ACT (exp) and PE (matmul) stay saturated; f32r for
the reciprocal-broadcast matmul; V-augmentation built by the transpose DMA
itself.

Self-contained: hardcodes B=2, S=2048, D=1024, H=16, d_k=64.
"""
import numpy as np
import ml_dtypes

B, S, D, H, DK = 2, 2048, 1024, 16, 64
NCORES = 8
HPC = H // NCORES            # 2 heads per core
THETA = 10000.0
BS = B * S                   # 4096 flattened tokens (b-major)
KT = D // 128                # 8 contraction tiles
P = 128
NCH = 4                      # x column chunks of 1024

bf16 = ml_dtypes.bfloat16

_CACHED_NC = None


def _host_prep(x, token_positions, W_qkv, W_o):
    """Build per-core DRAM input dicts (numpy, bf16), SBUF-layout contiguous."""
    cast = lambda a: np.ascontiguousarray(a).astype(bf16)
    X2T = np.asarray(x, np.float32).reshape(BS, D).T          # [D, BS]
    xt = cast(X2T.reshape(KT, P, NCH, 1024).transpose(1, 2, 0, 3))  # [P,4,KT,1024]

    pos = np.asarray(token_positions, np.float64)
    inv = THETA ** (-np.arange(0, DK, 2, dtype=np.float64) / DK)   # [32]
    ang = pos[:, None] * inv[None, :]                              # [S, 32]
    cosv = np.cos(ang).T.astype(np.float32)                        # [32, S]
    sinv = np.sin(ang).T.astype(np.float32)
    COS = cast(np.tile(cosv, (4, 1)))                              # [128, S]
    SINS = cast(np.concatenate([-sinv, sinv, -sinv, sinv], 0))     # [128, S]

    perm = np.concatenate([np.arange(0, 64, 2), np.arange(1, 64, 2)])
    tri = cast(np.triu(np.ones((P, P), np.float32)))               # [k,q]: q>=k

    sel4 = np.zeros((4, 4, 64), np.float32)
    for u in range(4):
        sel4[u, u, :] = 1.0                                        # lhsT rows=K

    def wlayout(w):        # [D, 128] -> [P, KT, 128] per-partition contiguous
        return cast(w.T.reshape(KT, P, P).transpose(1, 0, 2))

    Wqkv = np.asarray(W_qkv, np.float32)
    Wo = np.asarray(W_o, np.float32)
    maps = []
    for c in range(NCORES):
        hA = HPC * c
        rows = np.concatenate([(hA + 0) * 64 + perm, (hA + 1) * 64 + perm])
        rows_v = np.concatenate([(hA + 0) * 64 + np.arange(64),
                                 (hA + 1) * 64 + np.arange(64)])
        maps.append({
            "xt": xt,
            "wq": wlayout(Wqkv[rows]),
            "wk": wlayout(Wqkv[D + rows]),
            "wv": wlayout(Wqkv[2 * D + rows_v]),
            "wo": cast(Wo[:, P * c:P * c + P].T),                  # [128, 1024]
            "cos": COS,
            "sin": SINS,
            "tri": tri,
            "sel4": sel4,
        })
    return maps


def _build_nc():
    """Trace + compile the per-core Bass module (same program on all cores)."""
    from contextlib import ExitStack
    import concourse.bacc as bacc
    import concourse.mybir as mybir
    import concourse.tile as tile
    from concourse.bass import ts

    f32 = mybir.dt.float32
    f32r = mybir.dt.float32r
    bf = mybir.dt.bfloat16
    EXP = mybir.ActivationFunctionType.Exp

    nc = bacc.Bacc("TRN2", target_bir_lowering=False, debug=False,
                   enable_asserts=False)

    xt_d = nc.dram_tensor("xt", [P, NCH, KT, 1024], bf, kind="ExternalInput").ap()
    wq_d = nc.dram_tensor("wq", [P, KT, P], bf, kind="ExternalInput").ap()
    wk_d = nc.dram_tensor("wk", [P, KT, P], bf, kind="ExternalInput").ap()
    wv_d = nc.dram_tensor("wv", [P, KT, P], bf, kind="ExternalInput").ap()
    wo_d = nc.dram_tensor("wo", [P, D], bf, kind="ExternalInput").ap()
    cos_d = nc.dram_tensor("cos", [P, S], bf, kind="ExternalInput").ap()
    sin_d = nc.dram_tensor("sin", [P, S], bf, kind="ExternalInput").ap()
    tri_d = nc.dram_tensor("tri", [P, P], bf, kind="ExternalInput").ap()
    sel_d = nc.dram_tensor("sel4", [4, 4, 64], f32r, kind="ExternalInput").ap()
    yt_d = nc.dram_tensor("yt", [P, 8, 8, 512], bf, kind="ExternalOutput").ap()

    with tile.TileContext(nc) as tc, ExitStack() as ctx:
        # ---- kernel-lifetime tiles / pools ----
        pp = ctx.enter_context(tc.tile_pool(name="persist", bufs=1))
        WO = pp.tile([P, D], bf, tag="wo")
        TRI = pp.tile([P, P], bf, tag="tri")
        SEL4 = pp.tile([4, 4, 64], f32r, tag="sel4")
        SCR = pp.tile([P, 8], bf, tag="scr")
        # attention working buffers
        ab = ctx.enter_context(tc.tile_pool(name="attnbuf", bufs=1))
        QA = ab.tile([P, BS], bf, tag="qa")
        KA = ab.tile([P, BS], bf, tag="ka")
        VT = ab.tile([P, BS], bf, tag="vt")
        VAB = [ab.tile([P, 16, 132], bf, tag=f"vab{b}", name=f"vab{b}")
               for b in range(B)]
        # cycling pools (slots shared per tag)
        oap = ctx.enter_context(tc.tile_pool(name="oaccp", bufs=1))
        srp = ctx.enter_context(tc.tile_pool(name="sums", bufs=1))
        opp = ctx.enter_context(tc.tile_pool(name="oprp", bufs=2))
        stg = ctx.enter_context(tc.tile_pool(name="stage", bufs=2))
        ybp = ctx.enter_context(tc.tile_pool(name="ybig", bufs=2))
        ptA0 = ctx.enter_context(tc.tile_pool(name="pta0", bufs=8))
        ptA1 = ctx.enter_context(tc.tile_pool(name="pta1", bufs=8))
        ptB0 = ctx.enter_context(tc.tile_pool(name="ptb0", bufs=8))
        ptB1 = ctx.enter_context(tc.tile_pool(name="ptb1", bufs=8))
        scp = ctx.enter_context(tc.tile_pool(name="scps", bufs=2, space="PSUM"))

        nc.gpsimd.memset(SCR[:], 0.0)

        pts = {}
        OACC = [None, None]

        def scores_group(b, g, pools=None, irange=None):
            for i in irange or range(8 * g, 8 * g + 8):
                qs_i = 512 * (i // 4)
                qext = S - qs_i
                blk = b * S + 128 * i
                for h in range(HPC):
                    hsl = slice(64 * h, 64 * h + 64)
                    pool = (pools or (ptA0, ptA1, ptB0, ptB1))[i // 4]
                    ptw = (2048, 1536, 1024, 512)[i // 4]
                    pt = pool.tile([P, ptw], bf, tag="pt")
                    off = 0
                    while off < qext:
                        w = min(1024, qext - off)
                        ps = scp.tile([P, 1024], f32, tag="sc")
                        vf = max(0, 128 * i - (qs_i + off))
                        for qc in range(0, w, 512):
                            sub = min(512, w - qc)
                            q0 = qs_i + off + qc
                            nc.tensor.matmul(
                                ps[:, qc:qc + sub],
                                lhsT=KA[hsl, blk:blk + 128],
                                rhs=QA[hsl, b * S + q0:b * S + q0 + sub],
                                start=True, stop=True)
                        if vf < w:
                            nc.scalar.activation(
                                pt[:, off + vf:off + w],
                                ps[:, vf:w], EXP, scale=0.125)
                        if vf > 0:
                            nc.gpsimd.memset(pt[:, off:off + vf], 0.0)
                        off += w
                    dc = 128 * i - qs_i
                    nc.gpsimd.tensor_mul(pt[:, dc:dc + 128],
                                         pt[:, dc:dc + 128], TRI[:])
                    pts[(b, i, h)] = pt

        def emit_av(b, jlist, avp):
            # one PSUM accumulation group per (j, h) over all causal k-blocks
            if OACC[b] is None:
                OACC[b] = oap.tile([65, 8, 512], f32, tag="oacc",
                                   name=f"oacc{b}")
            for j in jlist:
                ilist = [i for i in range(16) if i <= 4 * j + 3]
                for h in range(HPC):
                    pa = avp.tile([65, 512], f32, tag="av")
                    for n, i in enumerate(ilist):
                        qs_i = 512 * (i // 4)
                        o0 = 512 * j - qs_i
                        nc.tensor.matmul(
                            pa[:],
                            lhsT=VAB[b][:, i, 65 * h:65 * h + 65],
                            rhs=pts[(b, i, h)][:, o0:o0 + 512],
                            start=(n == 0),
                            stop=(n == len(ilist) - 1))
                    u8 = j * 2 + h
                    nc.vector.tensor_copy(OACC[b][:, u8, :], pa[:])

        def div_oproj(b, jlist, avp, yps, tail=False):
            u0 = 2 * jlist[0]
            R = 2 * len(jlist)
            SUt = srp.tile([4, 512], f32, tag="sums")
            REf = srp.tile([4, 512], f32, tag="recip")
            REtt = srp.tile([4, 512], f32r, tag="recipr")
            SU, RE, REt = SUt[0:R, :], REf[0:R, :], REtt[0:R, :]
            nc.gpsimd.dma_start(SU, OACC[b][64:65, u0:u0 + R, :])
            nc.vector.reciprocal_approx_fast(RE, SU)
            nc.vector.tensor_copy(REt, RE)
            REr = REt
            for j in jlist:
                jj = b * 4 + j
                OPR = opp.tile([P, 512], bf, tag="opr")
                for h in range(HPC):
                    u8 = j * 2 + h
                    u4 = 2 * (j - jlist[0]) + h
                    pbt = avp.tile([65, 512], f32, tag="av", name="pbt")
                    pb = pbt[0:64, :]
                    nc.tensor.matmul(pb, lhsT=SEL4[0:R, u4, :], rhs=REr,
                                     start=True, stop=True)
                    if h == 0:
                        nc.vector.tensor_mul(OPR[0:64, :],
                                             OACC[b][0:64, u8, :], pb)
                    else:
                        tb = stg.tile([64, 512], bf, tag="tmpb")
                        nc.vector.tensor_mul(tb[:],
                                             OACC[b][0:64, u8, :], pb)
                        nc.gpsimd.dma_start(OPR[64:128, :], tb[:])
                for eh in range(2):
                    yb = ybp.tile([P, 4, 512], bf, tag="yb")
                    for e4 in range(4):
                        et = eh * 4 + e4
                        py = yps.tile([P, 512], f32, tag="y")
                        nc.tensor.matmul(py[:], lhsT=WO[:, ts(et, P)],
                                         rhs=OPR[:], start=True, stop=True)
                        if tail and et % 2 == 1:
                            nc.scalar.copy(yb[:, e4, :], py[:])
                        else:
                            nc.vector.tensor_copy(yb[:, e4, :], py[:])
                    nc.sync.dma_start(yt_d[:, jj, 4 * eh:4 * eh + 4, :], yb[:])

        # ---- QKV phase: 1024-col chunks, rope fused per chunk ----
        with tc.tile_pool(name="xp", bufs=2) as xp, \
             tc.tile_pool(name="wp", bufs=1) as wp, \
             tc.tile_pool(name="swap", bufs=2) as swp, \
             tc.tile_pool(name="vbp", bufs=2) as vbp, \
             tc.tile_pool(name="qkvps", bufs=2, space="PSUM") as qps:
            WQ = wp.tile([P, KT, P], bf, tag="wq")
            WK = wp.tile([P, KT, P], bf, tag="wk")
            WV = wp.tile([P, KT, P], bf, tag="wv")
            COS = wp.tile([P, S], bf, tag="cos")
            SIN = wp.tile([P, S], bf, tag="sin")
            nc.scalar.dma_start(WQ[:], wq_d)
            nc.scalar.dma_start(WK[:], wk_d)
            nc.scalar.dma_start(COS[:], cos_d)
            nc.scalar.dma_start(SIN[:], sin_d)
            nc.scalar.dma_start(WV[:], wv_d)
            nc.scalar.dma_start(WO[:], wo_d)
            nc.scalar.dma_start(TRI[:], tri_d)
            nc.scalar.dma_start(SEL4[:], sel_d)
            # preload the exp table set (after the weight DMAs on this queue)
            nc.scalar.activation(SCR[:], SCR[:], EXP)
            XT = []
            for c in range(NCH):
                xtc = xp.tile([P, KT, 1024], bf, tag="xt")
                # two half-chunk DMAs so the first matmuls start sooner
                nc.sync.dma_start(xtc[:, :, 0:512], xt_d[:, c, :, 0:512])
                nc.sync.dma_start(xtc[:, :, 512:1024], xt_d[:, c, :, 512:1024])
                XT.append(xtc)

            def project_chunk(Wt, DST, c, evac):
                ps = qps.tile([P, 1024], f32, tag="qkv")
                for jj in range(2):
                    for kt in range(KT):
                        nc.tensor.matmul(
                            ps[:, ts(jj, 512)], lhsT=Wt[:, kt, :],
                            rhs=XT[c][:, kt, ts(jj, 512)],
                            start=(kt == 0), stop=(kt == KT - 1))
                col = 1024 * c
                if evac == "s":
                    nc.scalar.copy(DST[:, col:col + 1024], ps[:])
                else:
                    nc.vector.tensor_copy(DST[:, col:col + 1024], ps[:])

            def rope_chunk(c):
                ssl = slice(1024 * c, 1024 * c + 1024)
                csl = slice(1024 * (c % 2), 1024 * (c % 2) + 1024)
                for A, tag in ((QA, "qs"), (KA, "ks")):
                    SWT = swp.tile([P, 1024], bf, tag=tag)
                    for blk in range(4):
                        src = blk ^ 1
                        nc.gpsimd.dma_start(SWT[32 * blk:32 * blk + 32, :],
                                            A[32 * src:32 * src + 32, ssl])
                    nc.vector.tensor_mul(A[:, ssl], A[:, ssl], COS[:, csl])
                    nc.vector.tensor_mul(SWT[:], SWT[:], SIN[:, csl])
                    nc.vector.tensor_add(A[:, ssl], A[:, ssl], SWT[:])

            def vaug(b, vbp):
                # HW xbar transpose cannot write a strided dst: go through a
                # dense tile, then gpsimd-copy into the augmented layout.
                for half, dst0 in ((0, 0), (1, 65)):
                    vb = vbp.tile([P, 16, 64], bf, tag="vb")
                    nc.sync.dma_start_transpose(
                        vb[:], VT[64 * half:64 * half + 64, b * S:(b + 1) * S])
                    nc.vector.tensor_copy(VAB[b][:, :, dst0:dst0 + 64], vb[:])
                for i in range(16):
                    nc.gpsimd.memset(VAB[b][:, i, 64:65], 1.0)
                    nc.gpsimd.memset(VAB[b][:, i, 129:130], 1.0)

            for c in (0, 1):
                project_chunk(WQ, QA, c, "s")
                project_chunk(WK, KA, c, "v")
                rope_chunk(c)
            project_chunk(WV, VT, 0, "s")
            project_chunk(WV, VT, 1, "s")
            vaug(0, vbp)
            scores_group(0, 0)            # b0 scores overlap b1 QKV
            for c in (2, 3):
                project_chunk(WQ, QA, c, "s")
                project_chunk(WK, KA, c, "v")
                rope_chunk(c)
            project_chunk(WV, VT, 2, "v")
            project_chunk(WV, VT, 3, "v")
            vaug(1, vbp)

        # ---- attention pipeline, batches interleaved ----
        with tc.tile_pool(name="avps", bufs=2, space="PSUM") as avp, \
             tc.tile_pool(name="yps", bufs=2, space="PSUM") as yps, \
             tc.tile_pool(name="pta0b", bufs=8) as ptA0b, \
             tc.tile_pool(name="pta1b", bufs=8) as ptA1b:
            emit_av(0, [0, 1], avp)        # needs only b0 g0 pts
            div_oproj(0, [0, 1], avp, yps)
            scores_group(0, 1)
            scores_group(1, 0, pools=(ptA0b, ptA1b, None, None))
            emit_av(0, [2, 3], avp)
            div_oproj(0, [2, 3], avp, yps)
            emit_av(1, [0, 1], avp)        # needs only b1 g0 pts
            div_oproj(1, [0, 1], avp, yps)
            scores_group(1, 1)
            emit_av(1, [2, 3], avp)
            div_oproj(1, [2, 3], avp, yps, tail=True)

    nc.compile()
    return nc


def get_nc():
    global _CACHED_NC
    if _CACHED_NC is None:
        _CACHED_NC = _build_nc()
    return _CACHED_NC


def run_on_hw(in_maps, **kwargs):
    from concourse.bass_utils import run_bass_kernel_spmd
    nc = get_nc()
    return run_bass_kernel_spmd(nc, in_maps, core_ids=list(range(NCORES)),
                                **kwargs)


def _assemble(res):
    acc = np.zeros((P, 8, 8, 512), np.float32)
    for r in res.results:
        acc += np.asarray(r["yt"]).astype(np.float32)
    y = acc.transpose(2, 0, 1, 3).reshape(D, BS)     # [et,p,jj,q] -> [D, BS]
    return np.ascontiguousarray(y.T).reshape(B, S, D).astype(np.float32)


def kernel(x, token_positions, W_qkv, W_o):
    in_maps = _host_prep(x, token_positions, W_qkv, W_o)
    res = run_on_hw(in_maps)
    return _assemble(res)


# revision 46
# speedup vs baseline: 1.0740x; 1.0023x over previous
"""Multi-head self-attention (RoPE, causal) Trainium2 Bass kernel.

Sharding: head-parallel across 8 NeuronCores. Core c owns heads {2c, 2c+1}
for both batch rows. Each core computes its heads' QKV projection, RoPE,
causal flash attention (scores kept transposed [k, q]), the per-head
softmax normalization, and a partial output projection against its 128
columns of W_o. The host sums the 8 partial projections (the "all-reduce")
and reshapes.

v2 scheduling: per-partition-contiguous DRAM layouts (fast DMA), QKV
processed in 1024-column chunks with RoPE fused per chunk, batch-0 scores
emitted while batch-1 QKV still projects, and the two batches' attention
software-pipelined so ACT (exp) and PE (matmul) stay saturated; f32r for
the reciprocal-broadcast matmul; V-augmentation built by the transpose DMA
itself.

Self-contained: hardcodes B=2, S=2048, D=1024, H=16, d_k=64.
"""
import numpy as np
import ml_dtypes

B, S, D, H, DK = 2, 2048, 1024, 16, 64
NCORES = 8
HPC = H // NCORES            # 2 heads per core
THETA = 10000.0
BS = B * S                   # 4096 flattened tokens (b-major)
KT = D // 128                # 8 contraction tiles
P = 128
NCH = 4                      # x column chunks of 1024

bf16 = ml_dtypes.bfloat16

_CACHED_NC = None


def _host_prep(x, token_positions, W_qkv, W_o):
    """Build per-core DRAM input dicts (numpy, bf16), SBUF-layout contiguous."""
    cast = lambda a: np.ascontiguousarray(a).astype(bf16)
    X2T = np.asarray(x, np.float32).reshape(BS, D).T          # [D, BS]
    xt = cast(X2T.reshape(KT, P, NCH, 1024).transpose(1, 2, 0, 3))  # [P,4,KT,1024]

    pos = np.asarray(token_positions, np.float64)
    inv = THETA ** (-np.arange(0, DK, 2, dtype=np.float64) / DK)   # [32]
    ang = pos[:, None] * inv[None, :]                              # [S, 32]
    cosv = np.cos(ang).T.astype(np.float32)                        # [32, S]
    sinv = np.sin(ang).T.astype(np.float32)
    COS = cast(np.tile(cosv, (4, 1)))                              # [128, S]
    SINS = cast(np.concatenate([-sinv, sinv, -sinv, sinv], 0))     # [128, S]

    perm = np.concatenate([np.arange(0, 64, 2), np.arange(1, 64, 2)])
    tri = cast(np.triu(np.ones((P, P), np.float32)))               # [k,q]: q>=k

    sel4 = np.zeros((4, 4, 64), np.float32)
    for u in range(4):
        sel4[u, u, :] = 1.0                                        # lhsT rows=K

    def wlayout(w):        # [D, 128] -> [P, KT, 128] per-partition contiguous
        return cast(w.T.reshape(KT, P, P).transpose(1, 0, 2))

    Wqkv = np.asarray(W_qkv, np.float32)
    Wo = np.asarray(W_o, np.float32)
    maps = []
    for c in range(NCORES):
        hA = HPC * c
        rows = np.concatenate([(hA + 0) * 64 + perm, (hA + 1) * 64 + perm])
        rows_v = np.concatenate([(hA + 0) * 64 + np.arange(64),
                                 (hA + 1) * 64 + np.arange(64)])
        maps.append({
            "xt": xt,
            "wq": wlayout(Wqkv[rows]),
            "wk": wlayout(Wqkv[D + rows]),
            "wv": wlayout(Wqkv[2 * D + rows_v]),
            "wo": cast(Wo[:, P * c:P * c + P].T),                  # [128, 1024]
            "cos": COS,
            "sin": SINS,
            "tri": tri,
            "sel4": sel4,
        })
    return maps


def _build_nc():
    """Trace + compile the per-core Bass module (same program on all cores)."""
    from contextlib import ExitStack
    import concourse.bacc as bacc
    import concourse.mybir as mybir
    import concourse.tile as tile
    from concourse.bass import ts

    f32 = mybir.dt.float32
    f32r = mybir.dt.float32r
    bf = mybir.dt.bfloat16
    EXP = mybir.ActivationFunctionType.Exp

    nc = bacc.Bacc("TRN2", target_bir_lowering=False, debug=False,
                   enable_asserts=False)

    xt_d = nc.dram_tensor("xt", [P, NCH, KT, 1024], bf, kind="ExternalInput").ap()
    wq_d = nc.dram_tensor("wq", [P, KT, P], bf, kind="ExternalInput").ap()
    wk_d = nc.dram_tensor("wk", [P, KT, P], bf, kind="ExternalInput").ap()
    wv_d = nc.dram_tensor("wv", [P, KT, P], bf, kind="ExternalInput").ap()
    wo_d = nc.dram_tensor("wo", [P, D], bf, kind="ExternalInput").ap()
    cos_d = nc.dram_tensor("cos", [P, S], bf, kind="ExternalInput").ap()
    sin_d = nc.dram_tensor("sin", [P, S], bf, kind="ExternalInput").ap()
    tri_d = nc.dram_tensor("tri", [P, P], bf, kind="ExternalInput").ap()
    sel_d = nc.dram_tensor("sel4", [4, 4, 64], f32r, kind="ExternalInput").ap()
    yt_d = nc.dram_tensor("yt", [P, 8, 8, 512], bf, kind="ExternalOutput").ap()

    with tile.TileContext(nc) as tc, ExitStack() as ctx:
        # ---- kernel-lifetime tiles / pools ----
        pp = ctx.enter_context(tc.tile_pool(name="persist", bufs=1))
        WO = pp.tile([P, D], bf, tag="wo")
        TRI = pp.tile([P, P], bf, tag="tri")
        SEL4 = pp.tile([4, 4, 64], f32r, tag="sel4")
        SCR = pp.tile([P, 8], bf, tag="scr")
        # attention working buffers
        ab = ctx.enter_context(tc.tile_pool(name="attnbuf", bufs=1))
        QA = ab.tile([P, BS], bf, tag="qa")
        KA = ab.tile([P, BS], bf, tag="ka")
        VT = ab.tile([P, BS], bf, tag="vt")
        VAB = [ab.tile([P, 16, 132], bf, tag=f"vab{b}", name=f"vab{b}")
               for b in range(B)]
        # cycling pools (slots shared per tag)
        oap = ctx.enter_context(tc.tile_pool(name="oaccp", bufs=1))
        srp = ctx.enter_context(tc.tile_pool(name="sums", bufs=1))
        opp = ctx.enter_context(tc.tile_pool(name="oprp", bufs=2))
        stg = ctx.enter_context(tc.tile_pool(name="stage", bufs=2))
        ybp = ctx.enter_context(tc.tile_pool(name="ybig", bufs=2))
        ptA0 = ctx.enter_context(tc.tile_pool(name="pta0", bufs=8))
        ptA1 = ctx.enter_context(tc.tile_pool(name="pta1", bufs=8))
        ptB0 = ctx.enter_context(tc.tile_pool(name="ptb0", bufs=8))
        ptB1 = ctx.enter_context(tc.tile_pool(name="ptb1", bufs=8))
        scp = ctx.enter_context(tc.tile_pool(name="scps", bufs=2, space="PSUM"))

        nc.gpsimd.memset(SCR[:], 0.0)

        pts = {}
        OACC = [None, None]

        def scores_group(b, g, pools=None, irange=None):
            for i in irange or range(8 * g, 8 * g + 8):
                qs_i = 512 * (i // 4)
                qext = S - qs_i
                blk = b * S + 128 * i
                for h in range(HPC):
                    hsl = slice(64 * h, 64 * h + 64)
                    pool = (pools or (ptA0, ptA1, ptB0, ptB1))[i // 4]
                    ptw = (2048, 1536, 1024, 512)[i // 4]
                    pt = pool.tile([P, ptw], bf, tag="pt")
                    off = 0
                    while off < qext:
                        w = min(1024, qext - off)
                        ps = scp.tile([P, 1024], f32, tag="sc")
                        vf = max(0, 128 * i - (qs_i + off))
                        for qc in range(0, w, 512):
                            sub = min(512, w - qc)
                            q0 = qs_i + off + qc
                            nc.tensor.matmul(
                                ps[:, qc:qc + sub],
                                lhsT=KA[hsl, blk:blk + 128],
                                rhs=QA[hsl, b * S + q0:b * S + q0 + sub],
                                start=True, stop=True)
                        if vf < w:
                            nc.scalar.activation(
                                pt[:, off + vf:off + w],
                                ps[:, vf:w], EXP, scale=0.125)
                        if vf > 0:
                            nc.gpsimd.memset(pt[:, off:off + vf], 0.0)
                        off += w
                    dc = 128 * i - qs_i
                    nc.gpsimd.tensor_mul(pt[:, dc:dc + 128],
                                         pt[:, dc:dc + 128], TRI[:])
                    pts[(b, i, h)] = pt

        def emit_av(b, jlist, avp):
            # one PSUM accumulation group per (j, h) over all causal k-blocks
            if OACC[b] is None:
                OACC[b] = oap.tile([65, 8, 512], f32, tag="oacc",
                                   name=f"oacc{b}")
            for j in jlist:
                ilist = [i for i in range(16) if i <= 4 * j + 3]
                for h in range(HPC):
                    pa = avp.tile([65, 512], f32, tag="av")
                    for n, i in enumerate(ilist):
                        qs_i = 512 * (i // 4)
                        o0 = 512 * j - qs_i
                        nc.tensor.matmul(
                            pa[:],
                            lhsT=VAB[b][:, i, 65 * h:65 * h + 65],
                            rhs=pts[(b, i, h)][:, o0:o0 + 512],
                            start=(n == 0),
                            stop=(n == len(ilist) - 1))
                    u8 = j * 2 + h
                    nc.vector.tensor_copy(OACC[b][:, u8, :], pa[:])

        def div_oproj(b, jlist, avp, yps, tail=False):
            u0 = 2 * jlist[0]
            R = 2 * len(jlist)
            SUt = srp.tile([4, 512], f32, tag="sums")
            REf = srp.tile([4, 512], f32, tag="recip")
            REtt = srp.tile([4, 512], f32r, tag="recipr")
            SU, RE, REt = SUt[0:R, :], REf[0:R, :], REtt[0:R, :]
            nc.gpsimd.dma_start(SU, OACC[b][64:65, u0:u0 + R, :])
            nc.vector.reciprocal_approx_fast(RE, SU)
            nc.vector.tensor_copy(REt, RE)
            REr = REt
            for j in jlist:
                jj = b * 4 + j
                OPR = opp.tile([P, 512], bf, tag="opr")
                for h in range(HPC):
                    u8 = j * 2 + h
                    u4 = 2 * (j - jlist[0]) + h
                    pbt = avp.tile([65, 512], f32, tag="av", name="pbt")
                    pb = pbt[0:64, :]
                    nc.tensor.matmul(pb, lhsT=SEL4[0:R, u4, :], rhs=REr,
                                     start=True, stop=True)
                    if h == 0:
                        nc.vector.tensor_mul(OPR[0:64, :],
                                             OACC[b][0:64, u8, :], pb)
                    else:
                        tb = stg.tile([64, 512], bf, tag="tmpb")
                        nc.vector.tensor_mul(tb[:],
                                             OACC[b][0:64, u8, :], pb)
                        nc.gpsimd.dma_start(OPR[64:128, :], tb[:])
                for eh in range(2):
                    yb = ybp.tile([P, 4, 512], bf, tag="yb")
                    for e4 in range(4):
                        et = eh * 4 + e4
                        py = yps.tile([P, 512], f32, tag="y")
                        nc.tensor.matmul(py[:], lhsT=WO[:, ts(et, P)],
                                         rhs=OPR[:], start=True, stop=True)
                        if tail and et % 2 == 1:
                            nc.scalar.copy(yb[:, e4, :], py[:])
                        else:
                            nc.vector.tensor_copy(yb[:, e4, :], py[:])
                    nc.sync.dma_start(yt_d[:, jj, 4 * eh:4 * eh + 4, :], yb[:])

        # ---- QKV phase: 1024-col chunks, rope fused per chunk ----
        with tc.tile_pool(name="xp", bufs=2) as xp, \
             tc.tile_pool(name="wp", bufs=1) as wp, \
             tc.tile_pool(name="swap", bufs=2) as swp, \
             tc.tile_pool(name="vbp", bufs=2) as vbp, \
             tc.tile_pool(name="qkvps", bufs=2, space="PSUM") as qps:
            WQ = wp.tile([P, KT, P], bf, tag="wq")
            WK = wp.tile([P, KT, P], bf, tag="wk")
            WV = wp.tile([P, KT, P], bf, tag="wv")
            COS = wp.tile([P, S], bf, tag="cos")
            SIN = wp.tile([P, S], bf, tag="sin")
            nc.scalar.dma_start(WQ[:], wq_d)
            nc.scalar.dma_start(WK[:], wk_d)
            nc.scalar.dma_start(COS[:], cos_d)
            nc.scalar.dma_start(SIN[:], sin_d)
            nc.scalar.dma_start(WV[:], wv_d)
            nc.scalar.dma_start(WO[:], wo_d)
            nc.scalar.dma_start(TRI[:], tri_d)
            nc.scalar.dma_start(SEL4[:], sel_d)
            # preload the exp table set (after the weight DMAs on this queue)
            nc.scalar.activation(SCR[:], SCR[:], EXP)
            XT = []
            for c in range(NCH):
                xtc = xp.tile([P, KT, 1024], bf, tag="xt")
                # two half-chunk DMAs so the first matmuls start sooner
                nc.sync.dma_start(xtc[:, :, 0:512], xt_d[:, c, :, 0:512])
                nc.sync.dma_start(xtc[:, :, 512:1024], xt_d[:, c, :, 512:1024])
                XT.append(xtc)

            def project_chunk(Wt, DST, c, evac):
                ps = qps.tile([P, 1024], f32, tag="qkv")
                for jj in range(2):
                    for kt in range(KT):
                        nc.tensor.matmul(
                            ps[:, ts(jj, 512)], lhsT=Wt[:, kt, :],
                            rhs=XT[c][:, kt, ts(jj, 512)],
                            start=(kt == 0), stop=(kt == KT - 1))
                col = 1024 * c
                if evac == "s":
                    nc.scalar.copy(DST[:, col:col + 1024], ps[:])
                else:
                    nc.vector.tensor_copy(DST[:, col:col + 1024], ps[:])

            def rope_chunk(c):
                ssl = slice(1024 * c, 1024 * c + 1024)
                csl = slice(1024 * (c % 2), 1024 * (c % 2) + 1024)
                for A, tag in ((QA, "qs"), (KA, "ks")):
                    SWT = swp.tile([P, 1024], bf, tag=tag)
                    for blk in range(4):
                        src = blk ^ 1
                        nc.gpsimd.dma_start(SWT[32 * blk:32 * blk + 32, :],
                                            A[32 * src:32 * src + 32, ssl])
                    nc.vector.tensor_mul(A[:, ssl], A[:, ssl], COS[:, csl])
                    nc.vector.tensor_mul(SWT[:], SWT[:], SIN[:, csl])
                    nc.vector.tensor_add(A[:, ssl], A[:, ssl], SWT[:])

            def vaug(b, vbp):
                # HW xbar transpose cannot write a strided dst: go through a
                # dense tile, then gpsimd-copy into the augmented layout.
                for half, dst0 in ((0, 0), (1, 65)):
                    vb = vbp.tile([P, 16, 64], bf, tag="vb")
                    nc.sync.dma_start_transpose(
                        vb[:], VT[64 * half:64 * half + 64, b * S:(b + 1) * S])
                    nc.vector.tensor_copy(VAB[b][:, :, dst0:dst0 + 64], vb[:])
                for i in range(16):
                    nc.gpsimd.memset(VAB[b][:, i, 64:65], 1.0)
                    nc.gpsimd.memset(VAB[b][:, i, 129:130], 1.0)

            for c in (0, 1):
                project_chunk(WQ, QA, c, "s")
                project_chunk(WK, KA, c, "v")
                rope_chunk(c)
            scores_group(0, 0)            # b0 scores: only need QK(c0,c1)
            project_chunk(WV, VT, 0, "s")
            project_chunk(WV, VT, 1, "s")
            vaug(0, vbp)
            for c in (2, 3):
                project_chunk(WQ, QA, c, "s")
                project_chunk(WK, KA, c, "v")
                rope_chunk(c)
            project_chunk(WV, VT, 2, "v")
            project_chunk(WV, VT, 3, "v")
            vaug(1, vbp)

        # ---- attention pipeline, batches interleaved ----
        with tc.tile_pool(name="avps", bufs=2, space="PSUM") as avp, \
             tc.tile_pool(name="yps", bufs=2, space="PSUM") as yps, \
             tc.tile_pool(name="pta0b", bufs=8) as ptA0b, \
             tc.tile_pool(name="pta1b", bufs=8) as ptA1b:
            emit_av(0, [0, 1], avp)        # needs only b0 g0 pts
            div_oproj(0, [0, 1], avp, yps)
            scores_group(0, 1)
            scores_group(1, 0, pools=(ptA0b, ptA1b, None, None))
            emit_av(0, [2, 3], avp)
            div_oproj(0, [2, 3], avp, yps)
            emit_av(1, [0, 1], avp)        # needs only b1 g0 pts
            div_oproj(1, [0, 1], avp, yps)
            scores_group(1, 1)
            emit_av(1, [2, 3], avp)
            div_oproj(1, [2, 3], avp, yps, tail=True)

    nc.compile()
    return nc


def get_nc():
    global _CACHED_NC
    if _CACHED_NC is None:
        _CACHED_NC = _build_nc()
    return _CACHED_NC


def run_on_hw(in_maps, **kwargs):
    from concourse.bass_utils import run_bass_kernel_spmd
    nc = get_nc()
    return run_bass_kernel_spmd(nc, in_maps, core_ids=list(range(NCORES)),
                                **kwargs)


def _assemble(res):
    acc = np.zeros((P, 8, 8, 512), np.float32)
    for r in res.results:
        acc += np.asarray(r["yt"]).astype(np.float32)
    y = acc.transpose(2, 0, 1, 3).reshape(D, BS)     # [et,p,jj,q] -> [D, BS]
    return np.ascontiguousarray(y.T).reshape(B, S, D).astype(np.float32)


def kernel(x, token_positions, W_qkv, W_o):
    in_maps = _host_prep(x, token_positions, W_qkv, W_o)
    res = run_on_hw(in_maps)
    return _assemble(res)


# revision 50
# speedup vs baseline: 1.1025x; 1.0265x over previous
"""Multi-head self-attention (RoPE, causal) Trainium2 Bass kernel.

Sharding: head-parallel across 8 NeuronCores. Core c owns heads {2c, 2c+1}
for both batch rows. Each core computes its heads' QKV projection, RoPE,
causal flash attention (scores kept transposed [k, q]), the per-head
softmax normalization, and a partial output projection against its 128
columns of W_o. The host sums the 8 partial projections (the "all-reduce")
and reshapes.

v2 scheduling: per-partition-contiguous DRAM layouts (fast DMA), QKV
processed in 1024-column chunks with RoPE fused per chunk, batch-0 scores
emitted while batch-1 QKV still projects, and the two batches' attention
software-pipelined so ACT (exp) and PE (matmul) stay saturated; f32r for
the reciprocal-broadcast matmul; V-augmentation built by the transpose DMA
itself.

Self-contained: hardcodes B=2, S=2048, D=1024, H=16, d_k=64.
"""
import numpy as np
import ml_dtypes

B, S, D, H, DK = 2, 2048, 1024, 16, 64
NCORES = 8
HPC = H // NCORES            # 2 heads per core
THETA = 10000.0
BS = B * S                   # 4096 flattened tokens (b-major)
KT = D // 128                # 8 contraction tiles
P = 128
NCH = 4                      # x column chunks of 1024

bf16 = ml_dtypes.bfloat16

_CACHED_NC = None


def _host_prep(x, token_positions, W_qkv, W_o):
    """Build per-core DRAM input dicts (numpy, bf16), SBUF-layout contiguous."""
    cast = lambda a: np.ascontiguousarray(a).astype(bf16)
    X2T = np.asarray(x, np.float32).reshape(BS, D).T          # [D, BS]
    xt = cast(X2T.reshape(KT, P, NCH, 1024).transpose(1, 2, 0, 3))  # [P,4,KT,1024]

    pos = np.asarray(token_positions, np.float64)
    inv = THETA ** (-np.arange(0, DK, 2, dtype=np.float64) / DK)   # [32]
    ang = pos[:, None] * inv[None, :]                              # [S, 32]
    cosv = np.cos(ang).T.astype(np.float32)                        # [32, S]
    sinv = np.sin(ang).T.astype(np.float32)
    COS = cast(np.tile(cosv, (4, 1)))                              # [128, S]
    SINS = cast(np.concatenate([-sinv, sinv, -sinv, sinv], 0))     # [128, S]

    perm = np.concatenate([np.arange(0, 64, 2), np.arange(1, 64, 2)])
    tri = cast(np.triu(np.ones((P, P), np.float32)))               # [k,q]: q>=k

    sel4 = np.zeros((4, 4, 64), np.float32)
    for u in range(4):
        sel4[u, u, :] = 1.0                                        # lhsT rows=K

    def wlayout(w):        # [D, 128] -> [P, KT, 128] per-partition contiguous
        return cast(w.T.reshape(KT, P, P).transpose(1, 0, 2))

    Wqkv = np.asarray(W_qkv, np.float32)
    Wo = np.asarray(W_o, np.float32)
    maps = []
    for c in range(NCORES):
        hA = HPC * c
        rows = np.concatenate([(hA + 0) * 64 + perm, (hA + 1) * 64 + perm])
        rows_v = np.concatenate([(hA + 0) * 64 + np.arange(64),
                                 (hA + 1) * 64 + np.arange(64)])
        maps.append({
            "xt": xt,
            "wq": wlayout(Wqkv[rows]),
            "wk": wlayout(Wqkv[D + rows]),
            "wv": wlayout(Wqkv[2 * D + rows_v]),
            "wo": cast(Wo[:, P * c:P * c + P].T),                  # [128, 1024]
            "cos": COS,
            "sin": SINS,
            "tri": tri,
            "sel4": sel4,
        })
    return maps


def _build_nc():
    """Trace + compile the per-core Bass module (same program on all cores)."""
    from contextlib import ExitStack
    import concourse.bacc as bacc
    import concourse.mybir as mybir
    import concourse.tile as tile
    from concourse.bass import ts

    f32 = mybir.dt.float32
    f32r = mybir.dt.float32r
    bf = mybir.dt.bfloat16
    EXP = mybir.ActivationFunctionType.Exp

    nc = bacc.Bacc("TRN2", target_bir_lowering=False, debug=False,
                   enable_asserts=False)

    xt_d = nc.dram_tensor("xt", [P, NCH, KT, 1024], bf, kind="ExternalInput").ap()
    wq_d = nc.dram_tensor("wq", [P, KT, P], bf, kind="ExternalInput").ap()
    wk_d = nc.dram_tensor("wk", [P, KT, P], bf, kind="ExternalInput").ap()
    wv_d = nc.dram_tensor("wv", [P, KT, P], bf, kind="ExternalInput").ap()
    wo_d = nc.dram_tensor("wo", [P, D], bf, kind="ExternalInput").ap()
    cos_d = nc.dram_tensor("cos", [P, S], bf, kind="ExternalInput").ap()
    sin_d = nc.dram_tensor("sin", [P, S], bf, kind="ExternalInput").ap()
    tri_d = nc.dram_tensor("tri", [P, P], bf, kind="ExternalInput").ap()
    sel_d = nc.dram_tensor("sel4", [4, 4, 64], f32r, kind="ExternalInput").ap()
    yt_d = nc.dram_tensor("yt", [P, 8, 8, 512], bf, kind="ExternalOutput").ap()

    with tile.TileContext(nc) as tc, ExitStack() as ctx:
        # ---- kernel-lifetime tiles / pools ----
        pp = ctx.enter_context(tc.tile_pool(name="persist", bufs=1))
        WO = pp.tile([P, D], bf, tag="wo")
        TRI = pp.tile([P, P], bf, tag="tri")
        SEL4 = pp.tile([4, 4, 64], f32r, tag="sel4")
        SCR = pp.tile([P, 8], bf, tag="scr")
        # attention working buffers
        ab = ctx.enter_context(tc.tile_pool(name="attnbuf", bufs=1))
        QA = ab.tile([P, BS], bf, tag="qa")
        KA = ab.tile([P, BS], bf, tag="ka")
        VT = ab.tile([P, BS], bf, tag="vt")
        VAB = [ab.tile([P, 16, 132], bf, tag=f"vab{b}", name=f"vab{b}")
               for b in range(B)]
        # cycling pools (slots shared per tag)
        oap = ctx.enter_context(tc.tile_pool(name="oaccp", bufs=1))
        srp = ctx.enter_context(tc.tile_pool(name="sums", bufs=1))
        opp = ctx.enter_context(tc.tile_pool(name="oprp", bufs=2))
        stg = ctx.enter_context(tc.tile_pool(name="stage", bufs=2))
        ybp = ctx.enter_context(tc.tile_pool(name="ybig", bufs=2))
        ptA0 = ctx.enter_context(tc.tile_pool(name="pta0", bufs=8))
        ptA1 = ctx.enter_context(tc.tile_pool(name="pta1", bufs=8))
        ptB0 = ctx.enter_context(tc.tile_pool(name="ptb0", bufs=8))
        ptB1 = ctx.enter_context(tc.tile_pool(name="ptb1", bufs=8))
        scp = ctx.enter_context(tc.tile_pool(name="scps", bufs=2, space="PSUM"))

        nc.gpsimd.memset(SCR[:], 0.0)

        pts = {}
        OACC = [None, None]

        def scores_group(b, g, pools=None, irange=None):
            for i in irange or range(8 * g, 8 * g + 8):
                qs_i = 512 * (i // 4)
                qext = S - qs_i
                blk = b * S + 128 * i
                for h in range(HPC):
                    hsl = slice(64 * h, 64 * h + 64)
                    pool = (pools or (ptA0, ptA1, ptB0, ptB1))[i // 4]
                    ptw = (2048, 1536, 1024, 512)[i // 4]
                    pt = pool.tile([P, ptw], bf, tag="pt")
                    off = 0
                    while off < qext:
                        w = min(1024, qext - off)
                        ps = scp.tile([P, 1024], f32, tag="sc")
                        vf = max(0, 128 * i - (qs_i + off))
                        for qc in range(0, w, 512):
                            sub = min(512, w - qc)
                            q0 = qs_i + off + qc
                            nc.tensor.matmul(
                                ps[:, qc:qc + sub],
                                lhsT=KA[hsl, blk:blk + 128],
                                rhs=QA[hsl, b * S + q0:b * S + q0 + sub],
                                start=True, stop=True)
                        if vf < w:
                            nc.scalar.activation(
                                pt[:, off + vf:off + w],
                                ps[:, vf:w], EXP, scale=0.125)
                        if vf > 0:
                            nc.gpsimd.memset(pt[:, off:off + vf], 0.0)
                        off += w
                    dc = 128 * i - qs_i
                    eng = nc.vector if b == 1 else nc.gpsimd
                    eng.tensor_mul(pt[:, dc:dc + 128],
                                   pt[:, dc:dc + 128], TRI[:])
                    pts[(b, i, h)] = pt

        def emit_av(b, jlist, avp):
            # one PSUM accumulation group per (j, h) over all causal k-blocks
            if OACC[b] is None:
                OACC[b] = oap.tile([65, 8, 512], f32, tag="oacc",
                                   name=f"oacc{b}")
            for j in jlist:
                ilist = [i for i in range(16) if i <= 4 * j + 3]
                for h in range(HPC):
                    pa = avp.tile([65, 512], f32, tag="av")
                    for n, i in enumerate(ilist):
                        qs_i = 512 * (i // 4)
                        o0 = 512 * j - qs_i
                        nc.tensor.matmul(
                            pa[:],
                            lhsT=VAB[b][:, i, 65 * h:65 * h + 65],
                            rhs=pts[(b, i, h)][:, o0:o0 + 512],
                            start=(n == 0),
                            stop=(n == len(ilist) - 1))
                    u8 = j * 2 + h
                    nc.vector.tensor_copy(OACC[b][:, u8, :], pa[:])

        def div_oproj(b, jlist, avp, yps, tail=False):
            u0 = 2 * jlist[0]
            R = 2 * len(jlist)
            SUt = srp.tile([4, 512], f32, tag="sums")
            REf = srp.tile([4, 512], f32, tag="recip")
            REtt = srp.tile([4, 512], f32r, tag="recipr")
            SU, RE, REt = SUt[0:R, :], REf[0:R, :], REtt[0:R, :]
            nc.gpsimd.dma_start(SU, OACC[b][64:65, u0:u0 + R, :])
            nc.vector.reciprocal_approx_fast(RE, SU)
            nc.vector.tensor_copy(REt, RE)
            REr = REt
            for j in jlist:
                jj = b * 4 + j
                OPR = opp.tile([P, 512], bf, tag="opr")
                for h in range(HPC):
                    u8 = j * 2 + h
                    u4 = 2 * (j - jlist[0]) + h
                    pbt = avp.tile([65, 512], f32, tag="av", name="pbt")
                    pb = pbt[0:64, :]
                    nc.tensor.matmul(pb, lhsT=SEL4[0:R, u4, :], rhs=REr,
                                     start=True, stop=True)
                    if h == 0:
                        nc.vector.tensor_mul(OPR[0:64, :],
                                             OACC[b][0:64, u8, :], pb)
                    else:
                        tb = stg.tile([64, 512], bf, tag="tmpb")
                        nc.vector.tensor_mul(tb[:],
                                             OACC[b][0:64, u8, :], pb)
                        nc.gpsimd.dma_start(OPR[64:128, :], tb[:])
                for eh in range(2):
                    yb = ybp.tile([P, 4, 512], bf, tag="yb")
                    for e4 in range(4):
                        et = eh * 4 + e4
                        py = yps.tile([P, 512], f32, tag="y")
                        nc.tensor.matmul(py[:], lhsT=WO[:, ts(et, P)],
                                         rhs=OPR[:], start=True, stop=True)
                        if tail and et % 2 == 1:
                            nc.scalar.copy(yb[:, e4, :], py[:])
                        else:
                            nc.vector.tensor_copy(yb[:, e4, :], py[:])
                    nc.sync.dma_start(yt_d[:, jj, 4 * eh:4 * eh + 4, :], yb[:])

        # ---- QKV phase: 1024-col chunks, rope fused per chunk ----
        with tc.tile_pool(name="xp", bufs=2) as xp, \
             tc.tile_pool(name="wp", bufs=1) as wp, \
             tc.tile_pool(name="swap", bufs=2) as swp, \
             tc.tile_pool(name="vbp", bufs=2) as vbp, \
             tc.tile_pool(name="qkvps", bufs=2, space="PSUM") as qps:
            WQ = wp.tile([P, KT, P], bf, tag="wq")
            WK = wp.tile([P, KT, P], bf, tag="wk")
            WV = wp.tile([P, KT, P], bf, tag="wv")
            COS = wp.tile([P, S], bf, tag="cos")
            SIN = wp.tile([P, S], bf, tag="sin")
            nc.scalar.dma_start(WQ[:], wq_d)
            nc.scalar.dma_start(WK[:], wk_d)
            nc.scalar.dma_start(COS[:], cos_d)
            nc.scalar.dma_start(SIN[:], sin_d)
            nc.scalar.dma_start(WV[:], wv_d)
            nc.scalar.dma_start(WO[:], wo_d)
            nc.scalar.dma_start(TRI[:], tri_d)
            nc.scalar.dma_start(SEL4[:], sel_d)
            # preload the exp table set (after the weight DMAs on this queue)
            nc.scalar.activation(SCR[:], SCR[:], EXP)
            XT = []
            for c in range(NCH):
                xtc = xp.tile([P, KT, 1024], bf, tag="xt")
                # two half-chunk DMAs so the first matmuls start sooner
                nc.sync.dma_start(xtc[:, :, 0:512], xt_d[:, c, :, 0:512])
                nc.sync.dma_start(xtc[:, :, 512:1024], xt_d[:, c, :, 512:1024])
                XT.append(xtc)

            def project_chunk(Wt, DST, c, evac):
                ps = qps.tile([P, 1024], f32, tag="qkv")
                for jj in range(2):
                    for kt in range(KT):
                        nc.tensor.matmul(
                            ps[:, ts(jj, 512)], lhsT=Wt[:, kt, :],
                            rhs=XT[c][:, kt, ts(jj, 512)],
                            start=(kt == 0), stop=(kt == KT - 1))
                col = 1024 * c
                if evac == "s":
                    nc.scalar.copy(DST[:, col:col + 1024], ps[:])
                else:
                    nc.vector.tensor_copy(DST[:, col:col + 1024], ps[:])

            def rope_chunk(c):
                ssl = slice(1024 * c, 1024 * c + 1024)
                csl = slice(1024 * (c % 2), 1024 * (c % 2) + 1024)
                for A, tag in ((QA, "qs"), (KA, "ks")):
                    SWT = swp.tile([P, 1024], bf, tag=tag)
                    for blk in range(4):
                        src = blk ^ 1
                        nc.gpsimd.dma_start(SWT[32 * blk:32 * blk + 32, :],
                                            A[32 * src:32 * src + 32, ssl])
                    nc.vector.tensor_mul(A[:, ssl], A[:, ssl], COS[:, csl])
                    nc.vector.tensor_mul(SWT[:], SWT[:], SIN[:, csl])
                    nc.vector.tensor_add(A[:, ssl], A[:, ssl], SWT[:])

            def vaug(b, vbp):
                # HW xbar transpose cannot write a strided dst: go through a
                # dense tile, then gpsimd-copy into the augmented layout.
                for half, dst0 in ((0, 0), (1, 65)):
                    vb = vbp.tile([P, 16, 64], bf, tag="vb")
                    nc.sync.dma_start_transpose(
                        vb[:], VT[64 * half:64 * half + 64, b * S:(b + 1) * S])
                    nc.vector.tensor_copy(VAB[b][:, :, dst0:dst0 + 64], vb[:])
                for i in range(16):
                    nc.gpsimd.memset(VAB[b][:, i, 64:65], 1.0)
                    nc.gpsimd.memset(VAB[b][:, i, 129:130], 1.0)

            for c in (0, 1):
                project_chunk(WQ, QA, c, "s")
                project_chunk(WK, KA, c, "v")
                rope_chunk(c)
            project_chunk(WV, VT, 0, "s")
            project_chunk(WV, VT, 1, "s")
            vaug(0, vbp)
            scores_group(0, 0)            # b0 scores overlap b1 QKV
            for c in (2, 3):
                project_chunk(WQ, QA, c, "s")
                project_chunk(WK, KA, c, "v")
                rope_chunk(c)
            project_chunk(WV, VT, 2, "v")
            project_chunk(WV, VT, 3, "v")
            vaug(1, vbp)

        # ---- attention pipeline, batches interleaved ----
        with tc.tile_pool(name="avps", bufs=2, space="PSUM") as avp, \
             tc.tile_pool(name="yps", bufs=2, space="PSUM") as yps, \
             tc.tile_pool(name="pta0b", bufs=8) as ptA0b, \
             tc.tile_pool(name="pta1b", bufs=8) as ptA1b:
            emit_av(0, [0, 1], avp)        # needs only b0 g0 pts
            div_oproj(0, [0, 1], avp, yps)
            scores_group(0, 1)
            scores_group(1, 0, pools=(ptA0b, ptA1b, None, None))
            emit_av(0, [2, 3], avp)
            div_oproj(0, [2, 3], avp, yps)
            emit_av(1, [0, 1], avp)        # needs only b1 g0 pts
            div_oproj(1, [0, 1], avp, yps)
            scores_group(1, 1)
            emit_av(1, [2, 3], avp)
            div_oproj(1, [2, 3], avp, yps, tail=True)

    nc.compile()
    return nc


def get_nc():
    global _CACHED_NC
    if _CACHED_NC is None:
        _CACHED_NC = _build_nc()
    return _CACHED_NC


def run_on_hw(in_maps, **kwargs):
    from concourse.bass_utils import run_bass_kernel_spmd
    nc = get_nc()
    return run_bass_kernel_spmd(nc, in_maps, core_ids=list(range(NCORES)),
                                **kwargs)


def _assemble(res):
    acc = np.zeros((P, 8, 8, 512), np.float32)
    for r in res.results:
        acc += np.asarray(r["yt"]).astype(np.float32)
    y = acc.transpose(2, 0, 1, 3).reshape(D, BS)     # [et,p,jj,q] -> [D, BS]
    return np.ascontiguousarray(y.T).reshape(B, S, D).astype(np.float32)


def kernel(x, token_positions, W_qkv, W_o):
    in_maps = _host_prep(x, token_positions, W_qkv, W_o)
    res = run_on_hw(in_maps)
    return _assemble(res)
